# revision 5
# baseline (speedup 1.0000x reference)
"""Trainium2 Bass kernel for nn_ARMFeedForward (dense MoE w/ top-2 masked combine).

Sharding: data-parallel over tokens across 8 NeuronCores (1024 tokens/core),
weights replicated, no collectives. Host does layout/dtype prep only
(transpose + bf16 cast); all arithmetic of the module runs on-chip:
  logits = (x @ c_norm^T)/||x|| + x @ w_route^T          (fp32 on PE/DVE/ACT)
  gates  = top2-masked softmax(logits)                    (DVE/ACT)
  out    = sum_e gate_e * (gelu(x@W1_e + b1_e) @ W2_e + b2_e)   (bf16 PE, fp32 PSUM)

Structure: routing for all tokens runs in a two-pass prologue (sqrt-table
pass, then exp-table pass) so the ACT engine's piecewise-poly table is
switched ~3x per kernel instead of per tile; the FFN main loop then runs
gelu-only on ACT with PE saturated by bf16 matmuls.
"""

import sys
from contextlib import ExitStack

import numpy as np

try:
    import concourse  # noqa: F401
except ImportError:
    sys.path.insert(0, "/opt/trn_rl_repo")

import ml_dtypes

import concourse.bass as bass
import concourse.mybir as mybir
import concourse.tile as tile
from concourse import bacc, masks
from concourse.bass_utils import run_bass_kernel_spmd

F32 = mybir.dt.float32
BF16 = mybir.dt.bfloat16
AF = mybir.ActivationFunctionType
OP = mybir.AluOpType
AX = mybir.AxisListType

N_CORES = 8
B, S, D = 4, 2048, 1024
E, H = 8, 512
N_TOK = B * S              # 8192
T_CORE = N_TOK // N_CORES  # 1024 tokens per core
TT = 256                   # tokens per FFN tile
N_TILES = T_CORE // TT     # 4
NSL = T_CORE // 128        # 8 routing slices of 128 tokens
DC = D // 128              # 8 contraction chunks over d_model
HC = H // 128              # 4 chunks over expert hidden
NEG_BIG = -1.0e30
GELU_FUNC = AF.Gelu


def build_nc() -> bass.Bass:
    nc = bacc.Bacc("TRN2", target_bir_lowering=False, debug=False)

    # ---- DRAM parameters (per-core shard views, host-prepped layouts) ----
    xt32_d = nc.declare_dram_parameter("xt32", [128, DC, T_CORE], F32, isOutput=False)
    xt16_d = nc.declare_dram_parameter("xt16", [128, DC, T_CORE], BF16, isOutput=False)
    xn_d = nc.declare_dram_parameter("xn", [T_CORE, D], F32, isOutput=False)
    w1_d = nc.declare_dram_parameter("w1b", [E, 128, DC, H], BF16, isOutput=False)
    w2_d = nc.declare_dram_parameter("w2b", [E, 128, HC, D], BF16, isOutput=False)
    b1_d = nc.declare_dram_parameter("b1t", [128, E * HC], F32, isOutput=False)
    b2_d = nc.declare_dram_parameter("b2b", [E, D], BF16, isOutput=False)
    cent_d = nc.declare_dram_parameter("cent", [E, D], F32, isOutput=False)
    wrt_d = nc.declare_dram_parameter("wrt", [E, D], F32, isOutput=False)
    out_d = nc.declare_dram_parameter("out", [T_CORE, D], F32, isOutput=True)

    with tile.TileContext(nc) as tc:
        with ExitStack() as ctx:
            # ---------------- static SBUF tiles ----------------
            statics = ctx.enter_context(tc.tile_pool(name="statics", bufs=1))
            ident = statics.tile([128, 128], F32, tag="ident")
            ones1 = statics.tile([1, 128], BF16, tag="ones1")
            r_sb = statics.tile([128, DC, 2 * E], F32, tag="r_sb")  # [dP, dc, cos|rt]
            b1_sb = statics.tile([128, E * HC], F32, tag="b1_sb")
            b2_sb = statics.tile([E, D], BF16, tag="b2_sb")
            w1_sb = [
                statics.tile([128, DC, H], BF16, tag=f"w1_{e}", name=f"w1s_{e}")
                for e in range(E)
            ]
            w2_sb = [
                statics.tile([128, HC, D], BF16, tag=f"w2_{e}", name=f"w2s_{e}")
                for e in range(E)
            ]

            masks.make_identity(nc, ident[:, :])
            nc.vector.memset(ones1[:, :], 1.0)

            # ---------------- weight / constant loads ----------------
            nc.sync.dma_start(b1_sb[:, :], b1_d[:, :])
            nc.sync.dma_start(b2_sb[:, :], b2_d[:, :])
            for e in range(E):
                nc.sync.dma_start(w1_sb[e][:, :, :], w1_d[e, :, :, :])
                nc.sync.dma_start(w2_sb[e][:, :, :], w2_d[e, :, :, :])

            # ------------- centroid normalize + routing matrix R (transient) ----------
            with tc.tile_pool(name="pre", bufs=1) as pre_p, tc.tile_pool(
                name="pre_ps", bufs=2, space="PSUM"
            ) as pre_ps:
                cent_sb = pre_p.tile([E, D], F32, tag="cent_sb")
                wrt_sb = pre_p.tile([E, D], F32, tag="wrt_sb")
                csq_sb = pre_p.tile([E, D], F32, tag="csq_sb")
                cn2 = pre_p.tile([E, 1], F32, tag="cn2")
                crn = pre_p.tile([E, 1], F32, tag="crn")
                nc.sync.dma_start(cent_sb[:, :], cent_d[:, :])
                nc.sync.dma_start(wrt_sb[:, :], wrt_d[:, :])
                # c_norm = centroids / max(||centroids||, eps)
                nc.scalar.activation(
                    csq_sb[:, :], cent_sb[:, :], AF.Square, accum_out=cn2[:, :]
                )
                nc.scalar.activation(cn2[:, :], cn2[:, :], AF.Sqrt)
                nc.vector.tensor_scalar_max(cn2[:, :], cn2[:, :], 1.0e-12)
                nc.vector.reciprocal(crn[:, :], cn2[:, :])
                nc.vector.tensor_scalar(
                    cent_sb[:, :], cent_sb[:, :], crn[:, :], None, op0=OP.mult
                )
                # R[:, dc, 0:8] = c_norm^T chunk, R[:, dc, 8:16] = w_route^T chunk
                for dc in range(DC):
                    tp = pre_ps.tile([128, E], F32, tag="rtp", name=f"rtp_{dc}")
                    nc.tensor.transpose(
                        tp[:, :], cent_sb[:, bass.ts(dc, 128)], ident[0:E, 0:E]
                    )
                    nc.vector.tensor_copy(r_sb[:, dc, 0:E], tp[:, :])
                    tp2 = pre_ps.tile([128, E], F32, tag="rtp", name=f"rtp2_{dc}")
                    nc.tensor.transpose(
                        tp2[:, :], wrt_sb[:, bass.ts(dc, 128)], ident[0:E, 0:E]
                    )
                    nc.vector.tensor_copy(r_sb[:, dc, E : 2 * E], tp2[:, :])

            # ---------------- pools ----------------
            xt32_p = ctx.enter_context(tc.tile_pool(name="xt32", bufs=3))
            xt16_p = ctx.enter_context(tc.tile_pool(name="xt16", bufs=2))
            xn_p = ctx.enter_context(tc.tile_pool(name="xn", bufs=2))
            sq_p = ctx.enter_context(tc.tile_pool(name="sq", bufs=2))
            sm_p = ctx.enter_context(tc.tile_pool(name="smalls", bufs=2))
            rt_p = ctx.enter_context(tc.tile_pool(name="rt", bufs=NSL))  # lg/rinv live
            gt_p = ctx.enter_context(tc.tile_pool(name="gt", bufs=N_TILES))
            gf_p = ctx.enter_context(tc.tile_pool(name="gf", bufs=2))
            hg_p = ctx.enter_context(tc.tile_pool(name="hg", bufs=3))
            hs_p = ctx.enter_context(tc.tile_pool(name="hs", bufs=9))
            osb_p = ctx.enter_context(tc.tile_pool(name="osb", bufs=2))

            lp_ps = ctx.enter_context(tc.tile_pool(name="lp_ps", bufs=2, space="PSUM"))
            gt_ps = ctx.enter_context(tc.tile_pool(name="gt_ps", bufs=1, space="PSUM"))
            h_ps = ctx.enter_context(tc.tile_pool(name="h_ps", bufs=2, space="PSUM"))
            gbc_ps = ctx.enter_context(tc.tile_pool(name="gbc_ps", bufs=1, space="PSUM"))
            oa_ps = ctx.enter_context(tc.tile_pool(name="oa_ps", bufs=2, space="PSUM"))

            # ================= routing prologue =================
            # pass 1: norms (Square+Sqrt table) + fp32 logits matmuls -> lg
            lg_t, rinv_t = [], []
            for sl in range(NSL):
                ta = sl * 128
                xt32 = xt32_p.tile([128, DC, 128], F32, tag="xt32", name=f"xt32_{sl}")
                nc.sync.dma_start(xt32[:, :, :], xt32_d[:, :, ta : ta + 128])
                xnat = xn_p.tile([128, D], F32, tag="xnat", name=f"xn_{sl}")
                nc.sync.dma_start(xnat[:, :], xn_d[ta : ta + 128, :])

                sq = sq_p.tile([128, D], F32, tag="sq", name=f"sq_{sl}")
                n2 = sm_p.tile([128, 1], F32, tag="n2", name=f"n2_{sl}")
                nc.scalar.activation(sq[:, :], xnat[:, :], AF.Square, accum_out=n2[:, :])
                nc.scalar.activation(n2[:, :], n2[:, :], AF.Sqrt)
                nc.vector.tensor_scalar_max(n2[:, :], n2[:, :], 1.0e-12)
                rinv = rt_p.tile([128, 1], F32, tag="rinv", name=f"rinv_{sl}")
                nc.vector.reciprocal(rinv[:, :], n2[:, :])
                rinv_t.append(rinv)

                lps = lp_ps.tile([128, 2 * E], F32, tag="lps", name=f"lps_{sl}")
                for dc in range(DC):
                    nc.tensor.matmul(
                        lps[:, :],
                        xt32[:, dc, :],
                        r_sb[:, dc, :],
                        start=(dc == 0),
                        stop=(dc == DC - 1),
                    )
                lg = rt_p.tile([128, E], F32, tag="lg", name=f"lg_{sl}")
                nc.vector.tensor_scalar(
                    lg[:, :], lps[:, 0:E], rinv[:, :], None, op0=OP.mult
                )
                nc.vector.tensor_tensor(lg[:, :], lg[:, :], lps[:, E : 2 * E], op=OP.add)
                lg_t.append(lg)

            # pass 2: top-2 masked softmax (Exp table) + gate transpose
            gt16_t = [
                gt_p.tile([E, TT], BF16, tag="gt16", name=f"gt16_{ti}")
                for ti in range(N_TILES)
            ]
            for sl in range(NSL):
                lg = lg_t[sl]
                m1 = sm_p.tile([128, 1], F32, tag="m1", name=f"m1_{sl}")
                nc.vector.tensor_reduce(m1[:, :], lg[:, :], axis=AX.X, op=OP.max)
                nm1 = sm_p.tile([128, 1], F32, tag="nm1", name=f"nm1_{sl}")
                nc.vector.tensor_scalar(nm1[:, :], m1[:, :], -1.0, None, op0=OP.mult)
                ee = sm_p.tile([128, E], F32, tag="ee", name=f"ee_{sl}")
                nc.scalar.activation(ee[:, :], lg[:, :], AF.Exp, bias=nm1[:, :], scale=1.0)

                nm = sm_p.tile([128, E], F32, tag="nm", name=f"nm_{sl}")
                nc.vector.tensor_scalar(
                    nm[:, :], lg[:, :], m1[:, :], NEG_BIG, op0=OP.is_equal, op1=OP.mult
                )
                nc.vector.tensor_tensor(nm[:, :], lg[:, :], nm[:, :], op=OP.add)
                m2 = sm_p.tile([128, 1], F32, tag="m2", name=f"m2_{sl}")
                nc.vector.tensor_reduce(m2[:, :], nm[:, :], axis=AX.X, op=OP.max)

                gu = sm_p.tile([128, E], F32, tag="gu", name=f"gu_{sl}")
                nc.vector.tensor_scalar(gu[:, :], lg[:, :], m2[:, :], None, op0=OP.is_ge)
                nc.vector.tensor_tensor(gu[:, :], gu[:, :], ee[:, :], op=OP.mult)
                den = sm_p.tile([128, 1], F32, tag="den", name=f"den_{sl}")
                nc.vector.tensor_reduce(den[:, :], gu[:, :], axis=AX.X, op=OP.add)
                rden = sm_p.tile([128, 1], F32, tag="rden", name=f"rden_{sl}")
                nc.vector.reciprocal(rden[:, :], den[:, :])
                g = sm_p.tile([128, E], F32, tag="g", name=f"g_{sl}")
                nc.vector.tensor_scalar(g[:, :], gu[:, :], rden[:, :], None, op0=OP.mult)

                gtp = gt_ps.tile([E, 128], F32, tag="gtp", name=f"gtp_{sl}")
                nc.tensor.transpose(gtp[:, :], g[:, :], ident[:, :])
                nc.scalar.copy(gt16_t[sl // 2][:, bass.ts(sl % 2, 128)], gtp[:, :])

            gflat_t = []
            for ti in range(N_TILES):
                gflat = gf_p.tile([1, E, TT], BF16, tag="gflat", name=f"gflat_{ti}")
                nc.sync.dma_start(gflat[0:1, :, :], gt16_t[ti][:, :])
                gflat_t.append(gflat)

            # ================= FFN main loop =================
            for ti in range(N_TILES):
                t0 = ti * TT
                gt16 = gt16_t[ti]
                gflat = gflat_t[ti]

                xt16 = xt16_p.tile([128, DC, TT], BF16, tag="xt16", name=f"xt16_{ti}")
                nc.sync.dma_start(xt16[:, :, :], xt16_d[:, :, t0 : t0 + TT])

                hs_all = [
                    hs_p.tile([128, HC, TT], BF16, tag="hs", name=f"hs_{ti}_{e}")
                    for e in range(E)
                ]

                # ---------- phase A: per expert w1 -> gelu -> gate ----------
                for e in range(E):
                    gbc = gbc_ps.tile([128, TT], F32, tag="gbc", name=f"gbc_{ti}_{e}")
                    nc.tensor.matmul(
                        gbc[:, :], ones1[0:1, :], gflat[0:1, e, :], start=True, stop=True
                    )
                    for hc in range(HC):
                        hps = h_ps.tile([128, TT], F32, tag="hps", name=f"hps_{ti}_{e}_{hc}")
                        for dc in range(DC):
                            nc.tensor.matmul(
                                hps[:, :],
                                w1_sb[e][:, dc, bass.ts(hc, 128)],
                                xt16[:, dc, :],
                                start=(dc == 0),
                                stop=(dc == DC - 1),
                            )
                        hg = hg_p.tile([128, TT], F32, tag="hg", name=f"hg_{ti}_{e}_{hc}")
                        nc.scalar.activation(
                            hg[:, :],
                            hps[:, :],
                            GELU_FUNC,
                            bias=b1_sb[:, e * HC + hc : e * HC + hc + 1],
                        )
                        nc.vector.tensor_tensor(
                            hs_all[e][:, hc, :], hg[:, :], gbc[:, :], op=OP.mult
                        )

                # ---------- phase B: w2 + gated combine (+ gated b2) ----------
                for tsl in range(TT // 128):
                    ta = t0 + tsl * 128
                    oa = [
                        oa_ps.tile([128, 512], F32, tag="oa", name=f"oa_{ti}_{tsl}_{dh}")
                        for dh in range(2)
                    ]
                    for e in range(E):
                        for hc in range(HC):
                            for dh in range(2):
                                nc.tensor.matmul(
                                    oa[dh][:, :],
                                    hs_all[e][:, hc, bass.ts(tsl, 128)],
                                    w2_sb[e][:, hc, bass.ts(dh, 512)],
                                    start=(e == 0 and hc == 0),
                                    stop=False,
                                )
                    for dh in range(2):
                        nc.tensor.matmul(
                            oa[dh][:, :],
                            gt16[:, bass.ts(tsl, 128)],
                            b2_sb[:, bass.ts(dh, 512)],
                            start=False,
                            stop=True,
                        )
                        osb = osb_p.tile(
                            [128, 512], F32, tag="osb", name=f"osb_{ti}_{tsl}_{dh}"
                        )
                        nc.scalar.copy(osb[:, :], oa[dh][:, :])
                        nc.sync.dma_start(
                            out_d[ta : ta + 128, bass.ts(dh, 512)], osb[:, :]
                        )

    if not nc.is_finalized():
        nc.finalize()
    return nc


def _prep_inputs(x, w1, b1, w2, b2, centroids, w_route):
    """Host-side layout/dtype prep + sharding. Returns per-core in_maps."""
    bf16 = ml_dtypes.bfloat16
    xf = np.ascontiguousarray(x.reshape(N_TOK, D).astype(np.float32))
    w1b = np.ascontiguousarray(
        w1.astype(np.float32).reshape(E, DC, 128, H).transpose(0, 2, 1, 3).astype(bf16)
    )
    w2b = np.ascontiguousarray(
        w2.astype(np.float32).reshape(E, HC, 128, D).transpose(0, 2, 1, 3).astype(bf16)
    )
    b1t = np.ascontiguousarray(
        b1.astype(np.float32).reshape(E, HC, 128).transpose(2, 0, 1).reshape(128, E * HC)
    )
    b2b = np.ascontiguousarray(b2.astype(np.float32).astype(bf16))
    cent = np.ascontiguousarray(centroids.astype(np.float32))
    wrt = np.ascontiguousarray(w_route.astype(np.float32))

    in_maps = []
    for c in range(N_CORES):
        xs = xf[c * T_CORE : (c + 1) * T_CORE]            # [1024, 1024]
        xt = np.ascontiguousarray(xs.T)                    # [d, t]
        xt32 = np.ascontiguousarray(xt.reshape(DC, 128, T_CORE).transpose(1, 0, 2))
        in_maps.append(
            {
                "xt32": xt32,
                "xt16": np.ascontiguousarray(xt32.astype(bf16)),
                "xn": xs,
                "w1b": w1b,
                "w2b": w2b,
                "b1t": b1t,
                "b2b": b2b,
                "cent": cent,
                "wrt": wrt,
            }
        )
    return in_maps


_CACHE = {}


def kernel(**inputs) -> np.ndarray:
    in_maps = _prep_inputs(
        inputs["x"], inputs["w1"], inputs["b1"], inputs["w2"], inputs["b2"],
        inputs["centroids"], inputs["w_route"],
    )
    if "nc" not in _CACHE:
        _CACHE["nc"] = build_nc()
    res = run_bass_kernel_spmd(_CACHE["nc"], in_maps, core_ids=list(range(N_CORES)))
    out = np.concatenate([res.results[c]["out"] for c in range(N_CORES)], axis=0)
    return np.ascontiguousarray(out.reshape(B, S, D).astype(np.float32))


if __name__ == "__main__":
    rng = np.random.default_rng(0)
    ins = {
        "x": rng.standard_normal((B, S, D), dtype=np.float32),
        "w1": rng.standard_normal((E, D, H), dtype=np.float32) / np.sqrt(D),
        "b1": np.zeros((E, H), np.float32),
        "w2": rng.standard_normal((E, H, D), dtype=np.float32) / np.sqrt(H),
        "b2": np.zeros((E, D), np.float32),
        "centroids": rng.standard_normal((E, D), dtype=np.float32) * 0.02,
        "w_route": rng.standard_normal((E, D), dtype=np.float32),
    }
    out = kernel(**ins)
    print(out.shape, out.dtype)


# revision 7
# speedup vs baseline: 1.0304x; 1.0304x over previous
"""Trainium2 Bass kernel for nn_ARMFeedForward (dense MoE w/ top-2 masked combine).

Sharding: data-parallel over tokens across 8 NeuronCores (1024 tokens/core),
weights replicated, no collectives. Host does layout/dtype prep only
(transpose + bf16 cast); all arithmetic of the module runs on-chip:
  logits = (x @ c_norm^T)/||x|| + x @ w_route^T          (fp32 on PE/DVE/ACT)
  gates  = top2-masked softmax(logits)                    (DVE/ACT)
  out    = sum_e gate_e * (gelu(x@W1_e + b1_e) @ W2_e + b2_e)   (bf16 PE, fp32 PSUM)

Structure: routing for all tokens runs in a two-pass prologue (sqrt-table
pass, then exp-table pass) so the ACT engine's piecewise-poly table is
switched ~3x per kernel instead of per tile; the FFN main loop then runs
gelu-only on ACT with PE saturated by bf16 matmuls.
"""

import sys
from contextlib import ExitStack

import numpy as np

try:
    import concourse  # noqa: F401
except ImportError:
    sys.path.insert(0, "/opt/trn_rl_repo")

import ml_dtypes

import concourse.bass as bass
import concourse.mybir as mybir
import concourse.tile as tile
from concourse import bacc, masks
from concourse.bass_utils import run_bass_kernel_spmd

F32 = mybir.dt.float32
BF16 = mybir.dt.bfloat16
AF = mybir.ActivationFunctionType
OP = mybir.AluOpType
AX = mybir.AxisListType

N_CORES = 8
B, S, D = 4, 2048, 1024
E, H = 8, 512
N_TOK = B * S              # 8192
T_CORE = N_TOK // N_CORES  # 1024 tokens per core
TT = 512                   # tokens per FFN tile (N=512 matmuls hide LDWEIGHTS)
N_TILES = T_CORE // TT     # 2
NSL = T_CORE // 128        # 8 routing slices of 128 tokens
SPT = TT // 128            # routing slices per FFN tile (4)
DC = D // 128              # 8 contraction chunks over d_model
HC = H // 128              # 4 chunks over expert hidden
NEG_BIG = -1.0e30
GELU_FUNC = AF.Gelu


def build_nc() -> bass.Bass:
    nc = bacc.Bacc("TRN2", target_bir_lowering=False, debug=False)

    # ---- DRAM parameters (per-core shard views, host-prepped layouts) ----
    xt32_d = nc.declare_dram_parameter("xt32", [128, DC, T_CORE], F32, isOutput=False)
    xt16_d = nc.declare_dram_parameter("xt16", [128, DC, T_CORE], BF16, isOutput=False)
    xn_d = nc.declare_dram_parameter("xn", [T_CORE, D], F32, isOutput=False)
    w1_d = nc.declare_dram_parameter("w1b", [E, 128, DC, H], BF16, isOutput=False)
    w2_d = nc.declare_dram_parameter("w2b", [E, 128, HC, D], BF16, isOutput=False)
    b1_d = nc.declare_dram_parameter("b1t", [128, E * HC], F32, isOutput=False)
    b2_d = nc.declare_dram_parameter("b2b", [E, D], BF16, isOutput=False)
    cent_d = nc.declare_dram_parameter("cent", [E, D], F32, isOutput=False)
    wrt_d = nc.declare_dram_parameter("wrt", [E, D], F32, isOutput=False)
    out_d = nc.declare_dram_parameter("out", [T_CORE, D], F32, isOutput=True)

    with tile.TileContext(nc) as tc:
        with ExitStack() as ctx:
            # ---------------- static SBUF tiles ----------------
            statics = ctx.enter_context(tc.tile_pool(name="statics", bufs=1))
            ident = statics.tile([128, 128], F32, tag="ident")
            ones1 = statics.tile([1, 128], BF16, tag="ones1")
            r_sb = statics.tile([128, DC, 2 * E], F32, tag="r_sb")  # [dP, dc, cos|rt]
            b1_sb = statics.tile([128, E * HC], F32, tag="b1_sb")
            b2_sb = statics.tile([E, D], BF16, tag="b2_sb")
            w1_sb = [
                statics.tile([128, DC, H], BF16, tag=f"w1_{e}", name=f"w1s_{e}")
                for e in range(E)
            ]
            w2_sb = [
                statics.tile([128, HC, D], BF16, tag=f"w2_{e}", name=f"w2s_{e}")
                for e in range(E)
            ]

            masks.make_identity(nc, ident[:, :])
            nc.vector.memset(ones1[:, :], 1.0)

            # ---------------- weight / constant loads ----------------
            nc.sync.dma_start(b1_sb[:, :], b1_d[:, :])
            nc.sync.dma_start(b2_sb[:, :], b2_d[:, :])
            for e in range(E):
                nc.gpsimd.dma_start(w1_sb[e][:, :, :], w1_d[e, :, :, :])
                nc.gpsimd.dma_start(w2_sb[e][:, :, :], w2_d[e, :, :, :])

            # ------------- centroid normalize + routing matrix R (transient) ----------
            with tc.tile_pool(name="pre", bufs=1) as pre_p, tc.tile_pool(
                name="pre_ps", bufs=2, space="PSUM"
            ) as pre_ps:
                cent_sb = pre_p.tile([E, D], F32, tag="cent_sb")
                wrt_sb = pre_p.tile([E, D], F32, tag="wrt_sb")
                csq_sb = pre_p.tile([E, D], F32, tag="csq_sb")
                cn2 = pre_p.tile([E, 1], F32, tag="cn2")
                crn = pre_p.tile([E, 1], F32, tag="crn")
                nc.sync.dma_start(cent_sb[:, :], cent_d[:, :])
                nc.sync.dma_start(wrt_sb[:, :], wrt_d[:, :])
                # c_norm = centroids / max(||centroids||, eps)
                nc.scalar.activation(
                    csq_sb[:, :], cent_sb[:, :], AF.Square, accum_out=cn2[:, :]
                )
                nc.scalar.activation(cn2[:, :], cn2[:, :], AF.Sqrt)
                nc.vector.tensor_scalar_max(cn2[:, :], cn2[:, :], 1.0e-12)
                nc.vector.reciprocal(crn[:, :], cn2[:, :])
                nc.vector.tensor_scalar(
                    cent_sb[:, :], cent_sb[:, :], crn[:, :], None, op0=OP.mult
                )
                # R[:, dc, 0:8] = c_norm^T chunk, R[:, dc, 8:16] = w_route^T chunk
                for dc in range(DC):
                    tp = pre_ps.tile([128, E], F32, tag="rtp", name=f"rtp_{dc}")
                    nc.tensor.transpose(
                        tp[:, :], cent_sb[:, bass.ts(dc, 128)], ident[0:E, 0:E]
                    )
                    nc.vector.tensor_copy(r_sb[:, dc, 0:E], tp[:, :])
                    tp2 = pre_ps.tile([128, E], F32, tag="rtp", name=f"rtp2_{dc}")
                    nc.tensor.transpose(
                        tp2[:, :], wrt_sb[:, bass.ts(dc, 128)], ident[0:E, 0:E]
                    )
                    nc.vector.tensor_copy(r_sb[:, dc, E : 2 * E], tp2[:, :])

            # ---------------- pools ----------------
            xt32_p = ctx.enter_context(tc.tile_pool(name="xt32", bufs=2))
            xt16_p = ctx.enter_context(tc.tile_pool(name="xt16", bufs=1))
            xn_p = ctx.enter_context(tc.tile_pool(name="xn", bufs=1))
            sq_p = ctx.enter_context(tc.tile_pool(name="sq", bufs=1))
            sm_p = ctx.enter_context(tc.tile_pool(name="smalls", bufs=2))
            rt_p = ctx.enter_context(tc.tile_pool(name="rt", bufs=NSL))  # lg/rinv live
            gt_p = ctx.enter_context(tc.tile_pool(name="gt", bufs=N_TILES))
            gf_p = ctx.enter_context(tc.tile_pool(name="gf", bufs=1))
            hg_p = ctx.enter_context(tc.tile_pool(name="hg", bufs=2))
            hs_p = ctx.enter_context(tc.tile_pool(name="hs", bufs=8))
            osb_p = ctx.enter_context(tc.tile_pool(name="osb", bufs=2))

            lp_ps = ctx.enter_context(tc.tile_pool(name="lp_ps", bufs=2, space="PSUM"))
            gt_ps = ctx.enter_context(tc.tile_pool(name="gt_ps", bufs=1, space="PSUM"))
            h_ps = ctx.enter_context(tc.tile_pool(name="h_ps", bufs=2, space="PSUM"))
            gbc_ps = ctx.enter_context(tc.tile_pool(name="gbc_ps", bufs=1, space="PSUM"))
            oa_ps = ctx.enter_context(tc.tile_pool(name="oa_ps", bufs=2, space="PSUM"))

            # ================= routing prologue =================
            # pass 1: norms (Square+Sqrt table) + fp32 logits matmuls -> lg
            lg_t, rinv_t = [], []
            for sl in range(NSL):
                ta = sl * 128
                xt32 = xt32_p.tile([128, DC, 128], F32, tag="xt32", name=f"xt32_{sl}")
                nc.sync.dma_start(xt32[:, :, :], xt32_d[:, :, ta : ta + 128])
                xnat = xn_p.tile([128, D], F32, tag="xnat", name=f"xn_{sl}")
                nc.sync.dma_start(xnat[:, :], xn_d[ta : ta + 128, :])

                sq = sq_p.tile([128, D], F32, tag="sq", name=f"sq_{sl}")
                n2 = sm_p.tile([128, 1], F32, tag="n2", name=f"n2_{sl}")
                nc.scalar.activation(sq[:, :], xnat[:, :], AF.Square, accum_out=n2[:, :])
                nc.scalar.activation(n2[:, :], n2[:, :], AF.Sqrt)
                nc.vector.tensor_scalar_max(n2[:, :], n2[:, :], 1.0e-12)
                rinv = rt_p.tile([128, 1], F32, tag="rinv", name=f"rinv_{sl}")
                nc.vector.reciprocal(rinv[:, :], n2[:, :])
                rinv_t.append(rinv)

                lps = lp_ps.tile([128, 2 * E], F32, tag="lps", name=f"lps_{sl}")
                for dc in range(DC):
                    nc.tensor.matmul(
                        lps[:, :],
                        xt32[:, dc, :],
                        r_sb[:, dc, :],
                        start=(dc == 0),
                        stop=(dc == DC - 1),
                    )
                lg = rt_p.tile([128, E], F32, tag="lg", name=f"lg_{sl}")
                nc.vector.tensor_scalar(
                    lg[:, :], lps[:, 0:E], rinv[:, :], None, op0=OP.mult
                )
                nc.vector.tensor_tensor(lg[:, :], lg[:, :], lps[:, E : 2 * E], op=OP.add)
                lg_t.append(lg)

            # pass 2: top-2 masked softmax (Exp table) + gate transpose
            gt16_t = [
                gt_p.tile([E, TT], BF16, tag="gt16", name=f"gt16_{ti}")
                for ti in range(N_TILES)
            ]
            for sl in range(NSL):
                lg = lg_t[sl]
                m1 = sm_p.tile([128, 1], F32, tag="m1", name=f"m1_{sl}")
                nc.vector.tensor_reduce(m1[:, :], lg[:, :], axis=AX.X, op=OP.max)
                nm1 = sm_p.tile([128, 1], F32, tag="nm1", name=f"nm1_{sl}")
                nc.vector.tensor_scalar(nm1[:, :], m1[:, :], -1.0, None, op0=OP.mult)
                ee = sm_p.tile([128, E], F32, tag="ee", name=f"ee_{sl}")
                nc.scalar.activation(ee[:, :], lg[:, :], AF.Exp, bias=nm1[:, :], scale=1.0)

                nm = sm_p.tile([128, E], F32, tag="nm", name=f"nm_{sl}")
                nc.vector.tensor_scalar(
                    nm[:, :], lg[:, :], m1[:, :], NEG_BIG, op0=OP.is_equal, op1=OP.mult
                )
                nc.vector.tensor_tensor(nm[:, :], lg[:, :], nm[:, :], op=OP.add)
                m2 = sm_p.tile([128, 1], F32, tag="m2", name=f"m2_{sl}")
                nc.vector.tensor_reduce(m2[:, :], nm[:, :], axis=AX.X, op=OP.max)

                gu = sm_p.tile([128, E], F32, tag="gu", name=f"gu_{sl}")
                nc.vector.tensor_scalar(gu[:, :], lg[:, :], m2[:, :], None, op0=OP.is_ge)
                nc.vector.tensor_tensor(gu[:, :], gu[:, :], ee[:, :], op=OP.mult)
                den = sm_p.tile([128, 1], F32, tag="den", name=f"den_{sl}")
                nc.vector.tensor_reduce(den[:, :], gu[:, :], axis=AX.X, op=OP.add)
                rden = sm_p.tile([128, 1], F32, tag="rden", name=f"rden_{sl}")
                nc.vector.reciprocal(rden[:, :], den[:, :])
                g = sm_p.tile([128, E], F32, tag="g", name=f"g_{sl}")
                nc.vector.tensor_scalar(g[:, :], gu[:, :], rden[:, :], None, op0=OP.mult)

                gtp = gt_ps.tile([E, 128], F32, tag="gtp", name=f"gtp_{sl}")
                nc.tensor.transpose(gtp[:, :], g[:, :], ident[:, :])
                nc.scalar.copy(gt16_t[sl // SPT][:, bass.ts(sl % SPT, 128)], gtp[:, :])

            gflat_t = []
            for ti in range(N_TILES):
                gflat = gf_p.tile([1, E, TT], BF16, tag="gflat", name=f"gflat_{ti}")
                nc.sync.dma_start(gflat[0:1, :, :], gt16_t[ti][:, :])
                gflat_t.append(gflat)

            # ================= FFN main loop =================
            for ti in range(N_TILES):
                t0 = ti * TT
                gt16 = gt16_t[ti]
                gflat = gflat_t[ti]

                xt16 = xt16_p.tile([128, DC, TT], BF16, tag="xt16", name=f"xt16_{ti}")
                nc.scalar.dma_start(xt16[:, :, :], xt16_d[:, :, t0 : t0 + TT])

                hs_all = [
                    hs_p.tile([128, HC, TT], BF16, tag="hs", name=f"hs_{ti}_{e}")
                    for e in range(E)
                ]

                # ---------- phase A: per expert w1 -> gelu -> gate ----------
                for e in range(E):
                    gbc = gbc_ps.tile([128, TT], F32, tag="gbc", name=f"gbc_{ti}_{e}")
                    nc.tensor.matmul(
                        gbc[:, :], ones1[0:1, :], gflat[0:1, e, :], start=True, stop=True
                    )
                    for hc in range(HC):
                        hps = h_ps.tile([128, TT], F32, tag="hps", name=f"hps_{ti}_{e}_{hc}")
                        for dc in range(DC):
                            nc.tensor.matmul(
                                hps[:, :],
                                w1_sb[e][:, dc, bass.ts(hc, 128)],
                                xt16[:, dc, :],
                                start=(dc == 0),
                                stop=(dc == DC - 1),
                            )
                        hg = hg_p.tile([128, TT], F32, tag="hg", name=f"hg_{ti}_{e}_{hc}")
                        nc.scalar.activation(
                            hg[:, :],
                            hps[:, :],
                            GELU_FUNC,
                            bias=b1_sb[:, e * HC + hc : e * HC + hc + 1],
                        )
                        nc.vector.tensor_tensor(
                            hs_all[e][:, hc, :], hg[:, :], gbc[:, :], op=OP.mult
                        )

                # ---------- phase B: w2 + gated combine (+ gated b2) ----------
                for tsl in range(SPT):
                    ta = t0 + tsl * 128
                    oa = [
                        oa_ps.tile([128, 512], F32, tag="oa", name=f"oa_{ti}_{tsl}_{dh}")
                        for dh in range(2)
                    ]
                    for e in range(E):
                        for hc in range(HC):
                            for dh in range(2):
                                nc.tensor.matmul(
                                    oa[dh][:, :],
                                    hs_all[e][:, hc, bass.ts(tsl, 128)],
                                    w2_sb[e][:, hc, bass.ts(dh, 512)],
                                    start=(e == 0 and hc == 0),
                                    stop=False,
                                )
                    for dh in range(2):
                        nc.tensor.matmul(
                            oa[dh][:, :],
                            gt16[:, bass.ts(tsl, 128)],
                            b2_sb[:, bass.ts(dh, 512)],
                            start=False,
                            stop=True,
                        )
                        osb = osb_p.tile(
                            [128, 512], F32, tag="osb", name=f"osb_{ti}_{tsl}_{dh}"
                        )
                        nc.scalar.copy(osb[:, :], oa[dh][:, :])
                        nc.scalar.dma_start(
                            out_d[ta : ta + 128, bass.ts(dh, 512)], osb[:, :]
                        )

    if not nc.is_finalized():
        nc.finalize()
    return nc


def _prep_inputs(x, w1, b1, w2, b2, centroids, w_route):
    """Host-side layout/dtype prep + sharding. Returns per-core in_maps."""
    bf16 = ml_dtypes.bfloat16
    xf = np.ascontiguousarray(x.reshape(N_TOK, D).astype(np.float32))
    w1b = np.ascontiguousarray(
        w1.astype(np.float32).reshape(E, DC, 128, H).transpose(0, 2, 1, 3).astype(bf16)
    )
    w2b = np.ascontiguousarray(
        w2.astype(np.float32).reshape(E, HC, 128, D).transpose(0, 2, 1, 3).astype(bf16)
    )
    b1t = np.ascontiguousarray(
        b1.astype(np.float32).reshape(E, HC, 128).transpose(2, 0, 1).reshape(128, E * HC)
    )
    b2b = np.ascontiguousarray(b2.astype(np.float32).astype(bf16))
    cent = np.ascontiguousarray(centroids.astype(np.float32))
    wrt = np.ascontiguousarray(w_route.astype(np.float32))

    in_maps = []
    for c in range(N_CORES):
        xs = xf[c * T_CORE : (c + 1) * T_CORE]            # [1024, 1024]
        xt = np.ascontiguousarray(xs.T)                    # [d, t]
        xt32 = np.ascontiguousarray(xt.reshape(DC, 128, T_CORE).transpose(1, 0, 2))
        in_maps.append(
            {
                "xt32": xt32,
                "xt16": np.ascontiguousarray(xt32.astype(bf16)),
                "xn": xs,
                "w1b": w1b,
                "w2b": w2b,
                "b1t": b1t,
                "b2b": b2b,
                "cent": cent,
                "wrt": wrt,
            }
        )
    return in_maps


_CACHE = {}


def kernel(**inputs) -> np.ndarray:
    in_maps = _prep_inputs(
        inputs["x"], inputs["w1"], inputs["b1"], inputs["w2"], inputs["b2"],
        inputs["centroids"], inputs["w_route"],
    )
    if "nc" not in _CACHE:
        _CACHE["nc"] = build_nc()
    res = run_bass_kernel_spmd(_CACHE["nc"], in_maps, core_ids=list(range(N_CORES)))
    out = np.concatenate([res.results[c]["out"] for c in range(N_CORES)], axis=0)
    return np.ascontiguousarray(out.reshape(B, S, D).astype(np.float32))


if __name__ == "__main__":
    rng = np.random.default_rng(0)
    ins = {
        "x": rng.standard_normal((B, S, D), dtype=np.float32),
        "w1": rng.standard_normal((E, D, H), dtype=np.float32) / np.sqrt(D),
        "b1": np.zeros((E, H), np.float32),
        "w2": rng.standard_normal((E, H, D), dtype=np.float32) / np.sqrt(H),
        "b2": np.zeros((E, D), np.float32),
        "centroids": rng.standard_normal((E, D), dtype=np.float32) * 0.02,
        "w_route": rng.standard_normal((E, D), dtype=np.float32),
    }
    out = kernel(**ins)
    print(out.shape, out.dtype)


# revision 12
# speedup vs baseline: 1.0456x; 1.0147x over previous
"""Trainium2 Bass kernel for nn_ARMFeedForward (dense MoE w/ top-2 masked combine).

Sharding: data-parallel over tokens across 8 NeuronCores (1024 tokens/core),
weights replicated, no collectives. Host does layout/dtype prep only
(transpose + bf16 cast); all arithmetic of the module runs on-chip:
  logits = (x @ c_norm^T)/||x|| + x @ w_route^T          (fp32 on PE/DVE/ACT)
  gates  = top2-masked softmax(logits)                    (DVE/ACT)
  out    = sum_e gate_e * (gelu(x@W1_e + b1_e) @ W2_e + b2_e)   (bf16 PE, fp32 PSUM)

Schedule: per FFN tile (512 tokens), routing runs just ahead of the FFN so
tile-1 routing hides under tile-0 FFN compute. DMA traffic is split across
the three trigger queues (weights on the GpSimd SWDGE queue, x/out on the
SP HWDGE queue, xt16/gflat on the Act HWDGE queue) with fully-contiguous
descriptors so the critical prologue loads are not starved.
"""

import sys
from contextlib import ExitStack

import numpy as np

try:
    import concourse  # noqa: F401
except ImportError:
    sys.path.insert(0, "/opt/trn_rl_repo")

import ml_dtypes

import concourse.bass as bass
import concourse.mybir as mybir
import concourse.tile as tile
from concourse import bacc, masks
from concourse.bass_utils import run_bass_kernel_spmd

F32 = mybir.dt.float32
BF16 = mybir.dt.bfloat16
AF = mybir.ActivationFunctionType
OP = mybir.AluOpType
AX = mybir.AxisListType

N_CORES = 8
B, S, D = 4, 2048, 1024
E, H = 8, 512
N_TOK = B * S              # 8192
T_CORE = N_TOK // N_CORES  # 1024 tokens per core
TT = 512                   # tokens per FFN tile (N=512 matmuls hide LDWEIGHTS)
N_TILES = T_CORE // TT     # 2
NSL = T_CORE // 128        # 8 routing slices of 128 tokens
SPT = TT // 128            # routing slices per FFN tile (4)
DC = D // 128              # 8 contraction chunks over d_model
HC = H // 128              # 4 chunks over expert hidden
NEG_BIG = -1.0e30
GELU_FUNC = AF.Gelu


def build_nc() -> bass.Bass:
    nc = bacc.Bacc("TRN2", target_bir_lowering=False, debug=False)

    # ---- DRAM parameters (per-core shard views, host-prepped layouts) ----
    xt32_d = nc.declare_dram_parameter("xt32", [NSL, 128, DC, 128], F32, isOutput=False)
    xt16_d = nc.declare_dram_parameter("xt16", [N_TILES, 128, DC, TT], BF16, isOutput=False)
    xn_d = nc.declare_dram_parameter("xn", [T_CORE, D], F32, isOutput=False)
    w1_d = nc.declare_dram_parameter("w1b", [E, 128, DC, H], BF16, isOutput=False)
    w2_d = nc.declare_dram_parameter("w2b", [E, 128, HC, D], BF16, isOutput=False)
    b1_d = nc.declare_dram_parameter("b1t", [128, E * HC], F32, isOutput=False)
    b2_d = nc.declare_dram_parameter("b2b", [E, D], BF16, isOutput=False)
    cent_d = nc.declare_dram_parameter("cent", [E, D], F32, isOutput=False)
    wrt_d = nc.declare_dram_parameter("wrt", [E, D], F32, isOutput=False)
    out_d = nc.declare_dram_parameter("out", [T_CORE, D], F32, isOutput=True)

    with tile.TileContext(nc) as tc:
        with ExitStack() as ctx:
            # ---------------- static SBUF tiles ----------------
            statics = ctx.enter_context(tc.tile_pool(name="statics", bufs=1))
            ident = statics.tile([128, 128], F32, tag="ident")
            ones1 = statics.tile([1, 128], BF16, tag="ones1")
            r_sb = statics.tile([128, DC, 2 * E], F32, tag="r_sb")  # [dP, dc, cos|rt]
            b1_sb = statics.tile([128, E * HC], F32, tag="b1_sb")
            b2_sb = statics.tile([E, D], BF16, tag="b2_sb")
            w1_sb = [
                statics.tile([128, DC, H], BF16, tag=f"w1_{e}", name=f"w1s_{e}")
                for e in range(E)
            ]
            w2_sb = [
                statics.tile([128, HC, D], BF16, tag=f"w2_{e}", name=f"w2s_{e}")
                for e in range(E)
            ]

            masks.make_identity(nc, ident[:, :])
            nc.vector.memset(ones1[:, :], 1.0)

            # ------------- early DMA triggers -------------
            # xt16 on the Act HWDGE queue, triggered before any ACT compute.
            xt16_p = ctx.enter_context(tc.tile_pool(name="xt16", bufs=2))
            xt16_t = []
            for ti in range(N_TILES):
                xt16 = xt16_p.tile([128, DC, TT], BF16, tag="xt16", name=f"xt16_{ti}")
                nc.scalar.dma_start(xt16[:, :, :], xt16_d[ti, :, :, :])
                xt16_t.append(xt16)
            # weights on the GpSimd SWDGE queue (b1/b2 first: tiny, needed early)
            nc.gpsimd.dma_start(b1_sb[:, :], b1_d[:, :])
            nc.gpsimd.dma_start(b2_sb[:, :], b2_d[:, :])
            for e in range(E):
                nc.gpsimd.dma_start(w1_sb[e][:, :, :], w1_d[e, :, :, :])
                nc.gpsimd.dma_start(w2_sb[e][:, :, :], w2_d[e, :, :, :])

            # ------------- centroid normalize + routing matrix R (transient) ----------
            with tc.tile_pool(name="pre", bufs=1) as pre_p, tc.tile_pool(
                name="pre_ps", bufs=2, space="PSUM"
            ) as pre_ps:
                cent_sb = pre_p.tile([E, D], F32, tag="cent_sb")
                wrt_sb = pre_p.tile([E, D], F32, tag="wrt_sb")
                csq_sb = pre_p.tile([E, D], F32, tag="csq_sb")
                cn2 = pre_p.tile([E, 1], F32, tag="cn2")
                crn = pre_p.tile([E, 1], F32, tag="crn")
                nc.sync.dma_start(cent_sb[:, :], cent_d[:, :])
                nc.sync.dma_start(wrt_sb[:, :], wrt_d[:, :])
                # c_norm = centroids / max(||centroids||, eps)
                nc.scalar.activation(
                    csq_sb[:, :], cent_sb[:, :], AF.Square, accum_out=cn2[:, :]
                )
                nc.scalar.activation(cn2[:, :], cn2[:, :], AF.Sqrt)
                nc.vector.tensor_scalar_max(cn2[:, :], cn2[:, :], 1.0e-12)
                nc.vector.reciprocal(crn[:, :], cn2[:, :])
                nc.vector.tensor_scalar(
                    cent_sb[:, :], cent_sb[:, :], crn[:, :], None, op0=OP.mult
                )
                # R[:, dc, 0:8] = c_norm^T chunk, R[:, dc, 8:16] = w_route^T chunk
                for dc in range(DC):
                    tp = pre_ps.tile([128, E], F32, tag="rtp", name=f"rtp_{dc}")
                    nc.tensor.transpose(
                        tp[:, :], cent_sb[:, bass.ts(dc, 128)], ident[0:E, 0:E]
                    )
                    nc.vector.tensor_copy(r_sb[:, dc, 0:E], tp[:, :])
                    tp2 = pre_ps.tile([128, E], F32, tag="rtp", name=f"rtp2_{dc}")
                    nc.tensor.transpose(
                        tp2[:, :], wrt_sb[:, bass.ts(dc, 128)], ident[0:E, 0:E]
                    )
                    nc.vector.tensor_copy(r_sb[:, dc, E : 2 * E], tp2[:, :])

            # ---------------- pools ----------------
            xt32_p = ctx.enter_context(tc.tile_pool(name="xt32", bufs=2))
            xn_p = ctx.enter_context(tc.tile_pool(name="xn", bufs=1))
            sm_p = ctx.enter_context(tc.tile_pool(name="smalls", bufs=2))
            rt_p = ctx.enter_context(tc.tile_pool(name="rt", bufs=SPT + 1))
            gt_p = ctx.enter_context(tc.tile_pool(name="gt", bufs=1))
            gf_p = ctx.enter_context(tc.tile_pool(name="gf", bufs=1))
            hg_p = ctx.enter_context(tc.tile_pool(name="hg", bufs=2))
            hs_p = ctx.enter_context(tc.tile_pool(name="hs", bufs=8))
            osb_p = ctx.enter_context(tc.tile_pool(name="osb", bufs=2))

            lp_ps = ctx.enter_context(tc.tile_pool(name="lp_ps", bufs=2, space="PSUM"))
            gt_ps = ctx.enter_context(tc.tile_pool(name="gt_ps", bufs=1, space="PSUM"))
            h_ps = ctx.enter_context(tc.tile_pool(name="h_ps", bufs=2, space="PSUM"))
            gbc_ps = ctx.enter_context(tc.tile_pool(name="gbc_ps", bufs=1, space="PSUM"))
            oa_ps = ctx.enter_context(tc.tile_pool(name="oa_ps", bufs=2, space="PSUM"))

            # ================= per-tile: routing then FFN =================
            for ti in range(N_TILES):
                sl_lo, sl_hi = ti * SPT, (ti + 1) * SPT
                xt16 = xt16_t[ti]

                # ---- routing pass 1: norms + fp32 logits -> lg ----
                lg_t, rinv_t = [], []
                for sl in range(sl_lo, sl_hi):
                    ta = sl * 128
                    xt32 = xt32_p.tile(
                        [128, DC, 128], F32, tag="xt32", name=f"xt32_{sl}"
                    )
                    nc.sync.dma_start(xt32[:, :, :], xt32_d[sl, :, :, :])
                    xnat = xn_p.tile([128, D], F32, tag="xnat", name=f"xn_{sl}")
                    nc.sync.dma_start(xnat[:, :], xn_d[ta : ta + 128, :])

                    n2 = sm_p.tile([128, 1], F32, tag="n2", name=f"n2_{sl}")
                    nc.scalar.activation(
                        xnat[:, :], xnat[:, :], AF.Square, accum_out=n2[:, :]
                    )
                    nc.scalar.activation(n2[:, :], n2[:, :], AF.Sqrt)
                    nc.vector.tensor_scalar_max(n2[:, :], n2[:, :], 1.0e-12)
                    rinv = rt_p.tile([128, 1], F32, tag="rinv", name=f"rinv_{sl}")
                    nc.vector.reciprocal(rinv[:, :], n2[:, :])
                    rinv_t.append(rinv)

                    lps = lp_ps.tile([128, 2 * E], F32, tag="lps", name=f"lps_{sl}")
                    for dc in range(DC):
                        nc.tensor.matmul(
                            lps[:, :],
                            xt32[:, dc, :],
                            r_sb[:, dc, :],
                            start=(dc == 0),
                            stop=(dc == DC - 1),
                        )
                    lg = rt_p.tile([128, E], F32, tag="lg", name=f"lg_{sl}")
                    nc.vector.tensor_scalar(
                        lg[:, :], lps[:, 0:E], rinv[:, :], None, op0=OP.mult
                    )
                    nc.vector.tensor_tensor(
                        lg[:, :], lg[:, :], lps[:, E : 2 * E], op=OP.add
                    )
                    lg_t.append(lg)

                # ---- routing pass 2: top-2 masked softmax + gate transpose ----
                gt16 = gt_p.tile([E, TT], BF16, tag="gt16", name=f"gt16_{ti}")
                for k, sl in enumerate(range(sl_lo, sl_hi)):
                    lg = lg_t[k]
                    m1 = sm_p.tile([128, 1], F32, tag="m1", name=f"m1_{sl}")
                    nc.vector.tensor_reduce(m1[:, :], lg[:, :], axis=AX.X, op=OP.max)
                    nm1 = sm_p.tile([128, 1], F32, tag="nm1", name=f"nm1_{sl}")
                    nc.vector.tensor_scalar(nm1[:, :], m1[:, :], -1.0, None, op0=OP.mult)
                    ee = sm_p.tile([128, E], F32, tag="ee", name=f"ee_{sl}")
                    nc.scalar.activation(
                        ee[:, :], lg[:, :], AF.Exp, bias=nm1[:, :], scale=1.0
                    )

                    nm = sm_p.tile([128, E], F32, tag="nm", name=f"nm_{sl}")
                    nc.vector.tensor_scalar(
                        nm[:, :], lg[:, :], m1[:, :], NEG_BIG, op0=OP.is_equal, op1=OP.mult
                    )
                    nc.vector.tensor_tensor(nm[:, :], lg[:, :], nm[:, :], op=OP.add)
                    m2 = sm_p.tile([128, 1], F32, tag="m2", name=f"m2_{sl}")
                    nc.vector.tensor_reduce(m2[:, :], nm[:, :], axis=AX.X, op=OP.max)

                    gu = sm_p.tile([128, E], F32, tag="gu", name=f"gu_{sl}")
                    nc.vector.tensor_scalar(
                        gu[:, :], lg[:, :], m2[:, :], None, op0=OP.is_ge
                    )
                    nc.vector.tensor_tensor(gu[:, :], gu[:, :], ee[:, :], op=OP.mult)
                    den = sm_p.tile([128, 1], F32, tag="den", name=f"den_{sl}")
                    nc.vector.tensor_reduce(den[:, :], gu[:, :], axis=AX.X, op=OP.add)
                    rden = sm_p.tile([128, 1], F32, tag="rden", name=f"rden_{sl}")
                    nc.vector.reciprocal(rden[:, :], den[:, :])
                    g = sm_p.tile([128, E], F32, tag="g", name=f"g_{sl}")
                    nc.vector.tensor_scalar(
                        g[:, :], gu[:, :], rden[:, :], None, op0=OP.mult
                    )

                    gtp = gt_ps.tile([E, 128], F32, tag="gtp", name=f"gtp_{sl}")
                    nc.tensor.transpose(gtp[:, :], g[:, :], ident[:, :])
                    nc.scalar.copy(gt16[:, bass.ts(k, 128)], gtp[:, :])

                gflat = gf_p.tile([1, E, TT], BF16, tag="gflat", name=f"gflat_{ti}")
                nc.scalar.dma_start(gflat[0:1, :, :], gt16[:, :])

                # ---------- phase A: per expert w1 -> gelu -> gate ----------
                hs_all = [
                    hs_p.tile([128, HC, TT], BF16, tag="hs", name=f"hs_{ti}_{e}")
                    for e in range(E)
                ]
                for e in range(E):
                    gbc = gbc_ps.tile([128, TT], F32, tag="gbc", name=f"gbc_{ti}_{e}")
                    nc.tensor.matmul(
                        gbc[:, :], ones1[0:1, :], gflat[0:1, e, :], start=True, stop=True
                    )
                    for hc in range(HC):
                        hps = h_ps.tile(
                            [128, TT], F32, tag="hps", name=f"hps_{ti}_{e}_{hc}"
                        )
                        for dc in range(DC):
                            nc.tensor.matmul(
                                hps[:, :],
                                w1_sb[e][:, dc, bass.ts(hc, 128)],
                                xt16[:, dc, :],
                                start=(dc == 0),
                                stop=(dc == DC - 1),
                            )
                        hg = hg_p.tile(
                            [128, TT], BF16, tag="hg", name=f"hg_{ti}_{e}_{hc}"
                        )
                        nc.scalar.activation(
                            hg[:, :],
                            hps[:, :],
                            GELU_FUNC,
                            bias=b1_sb[:, e * HC + hc : e * HC + hc + 1],
                        )
                        nc.vector.tensor_tensor(
                            hs_all[e][:, hc, :], hg[:, :], gbc[:, :], op=OP.mult
                        )

                # ---------- phase B: w2 + gated combine (+ gated b2) ----------
                for tsl in range(SPT):
                    ta = ti * TT + tsl * 128
                    oa = [
                        oa_ps.tile([128, 512], F32, tag="oa", name=f"oa_{ti}_{tsl}_{dh}")
                        for dh in range(2)
                    ]
                    for e in range(E):
                        for hc in range(HC):
                            for dh in range(2):
                                nc.tensor.matmul(
                                    oa[dh][:, :],
                                    hs_all[e][:, hc, bass.ts(tsl, 128)],
                                    w2_sb[e][:, hc, bass.ts(dh, 512)],
                                    start=(e == 0 and hc == 0),
                                    stop=False,
                                )
                    for dh in range(2):
                        nc.tensor.matmul(
                            oa[dh][:, :],
                            gt16[:, bass.ts(tsl, 128)],
                            b2_sb[:, bass.ts(dh, 512)],
                            start=False,
                            stop=True,
                        )
                        osb = osb_p.tile(
                            [128, 512], F32, tag="osb", name=f"osb_{ti}_{tsl}_{dh}"
                        )
                        nc.scalar.copy(osb[:, :], oa[dh][:, :])
                        nc.sync.dma_start(
                            out_d[ta : ta + 128, bass.ts(dh, 512)], osb[:, :]
                        )

    if not nc.is_finalized():
        nc.finalize()
    return nc


def _prep_inputs(x, w1, b1, w2, b2, centroids, w_route):
    """Host-side layout/dtype prep + sharding. Returns per-core in_maps."""
    bf16 = ml_dtypes.bfloat16
    xf = np.ascontiguousarray(x.reshape(N_TOK, D).astype(np.float32))
    w1b = np.ascontiguousarray(
        w1.astype(np.float32).reshape(E, DC, 128, H).transpose(0, 2, 1, 3).astype(bf16)
    )
    w2b = np.ascontiguousarray(
        w2.astype(np.float32).reshape(E, HC, 128, D).transpose(0, 2, 1, 3).astype(bf16)
    )
    b1t = np.ascontiguousarray(
        b1.astype(np.float32).reshape(E, HC, 128).transpose(2, 0, 1).reshape(128, E * HC)
    )
    b2b = np.ascontiguousarray(b2.astype(np.float32).astype(bf16))
    cent = np.ascontiguousarray(centroids.astype(np.float32))
    wrt = np.ascontiguousarray(w_route.astype(np.float32))

    in_maps = []
    for c in range(N_CORES):
        xs = xf[c * T_CORE : (c + 1) * T_CORE]            # [1024, 1024]
        xt = np.ascontiguousarray(xs.T)                    # [d, t]
        # [NSL, 128dp, DC, 128t] — contiguous per routing slice
        xt32 = np.ascontiguousarray(
            xt.reshape(DC, 128, NSL, 128).transpose(2, 1, 0, 3)
        )
        # [N_TILES, 128dp, DC, TT] — contiguous per FFN tile
        xt16 = np.ascontiguousarray(
            xt.reshape(DC, 128, N_TILES, TT).transpose(2, 1, 0, 3).astype(bf16)
        )
        in_maps.append(
            {
                "xt32": xt32,
                "xt16": xt16,
                "xn": xs,
                "w1b": w1b,
                "w2b": w2b,
                "b1t": b1t,
                "b2b": b2b,
                "cent": cent,
                "wrt": wrt,
            }
        )
    return in_maps


_CACHE = {}


def kernel(**inputs) -> np.ndarray:
    in_maps = _prep_inputs(
        inputs["x"], inputs["w1"], inputs["b1"], inputs["w2"], inputs["b2"],
        inputs["centroids"], inputs["w_route"],
    )
    if "nc" not in _CACHE:
        _CACHE["nc"] = build_nc()
    res = run_bass_kernel_spmd(_CACHE["nc"], in_maps, core_ids=list(range(N_CORES)))
    out = np.concatenate([res.results[c]["out"] for c in range(N_CORES)], axis=0)
    return np.ascontiguousarray(out.reshape(B, S, D).astype(np.float32))


if __name__ == "__main__":
    rng = np.random.default_rng(0)
    ins = {
        "x": rng.standard_normal((B, S, D), dtype=np.float32),
        "w1": rng.standard_normal((E, D, H), dtype=np.float32) / np.sqrt(D),
        "b1": np.zeros((E, H), np.float32),
        "w2": rng.standard_normal((E, H, D), dtype=np.float32) / np.sqrt(H),
        "b2": np.zeros((E, D), np.float32),
        "centroids": rng.standard_normal((E, D), dtype=np.float32) * 0.02,
        "w_route": rng.standard_normal((E, D), dtype=np.float32),
    }
    out = kernel(**ins)
    print(out.shape, out.dtype)
